# revision 1
# baseline (speedup 1.0000x reference)
"""GNN message-passing layer on 8 Trainium2 NeuronCores.

Strategy (edge-type sharding, one edge type per core):
  core e: proj_e = node_states @ W[e].T + b[e]            (PE matmul, [N, D])
          gathered = proj_e[src[e]]                        (dma_gather, SWDGE)
          partial_e[tgt[e]] += gathered                    (dma_scatter_add)
  host:   messages = sum_e partial_e ; divide by bincount(tgt).

dma_gather / dma_scatter_add take int16 indices, so nodes are split into 4
buckets of 25000; edges are grouped host-side by (src_bucket, tgt_bucket)
into 16 groups and padded to a static chunk schedule (identical across all
cores: SPMD single program). Padding edges gather row 0 of the bucket and
scatter into per-bucket junk rows (88 spare rows after each target bucket).
"""

import numpy as np

import concourse.bacc as bacc
import concourse.bass as bass
import concourse.mybir as mybir
import concourse.tile as tile
from concourse.bass_utils import run_bass_kernel_spmd
from concourse.masks import make_identity

N = 100000   # nodes
D = 128      # hidden
E = 8        # edge types == cores
M = 200000   # edges per type

NB = 4         # node buckets (int16 index windows)
BS = 25000     # bucket size
JUNK = 88      # junk rows appended to each target bucket (absorb padding)
TBS = BS + JUNK
CHUNK = 1024   # max edges per gather/scatter call (HW SWDGE limit)
SUPER = 1024   # nodes per phase-A supertile
E_GROUPS = 16  # (src_bucket, tgt_bucket) groups

F32 = mybir.dt.float32
I16 = mybir.dt.int16

# test-harness knobs (harness calls kernel() with defaults)
TRACE = False
LAST = None


def build_schedule(edge_lists):
    """Group each core's edges by (src_bucket, tgt_bucket); within each group,
    deal a target's edges across different chunks (occurrence rank) so every
    dma_scatter_add call has unique target indices -- the HW DMA engines race
    on read-modify-write of duplicate rows within one call. In-degree > NMAIN
    within a group spills to small per-level overflow chunks.

    Returns (chunks, tot, gsrc_w, gtgt_w); chunks = (sb, tb, size, col_off).
    """
    assert edge_lists.shape == (E, M, 2)
    src = np.asarray(edge_lists[:, :, 0], dtype=np.int64)
    tgt = np.asarray(edge_lists[:, :, 1], dtype=np.int64)
    sb = src // BS
    tb = tgt // BS
    gid = sb * NB + tb                          # [E, M] group id 0..15
    NMAIN = max(4, -(-M // (E_GROUPS * (CHUNK - 160))))  # initial guess

    # occurrence rank of each edge within (core, group, target)
    occ = np.empty((E, M), dtype=np.int64)
    for e in range(E):
        key = gid[e] * (N + 1) + tgt[e]
        order = np.argsort(key, kind="stable")
        sk = key[order]
        run_start = np.empty(M, dtype=bool)
        run_start[0] = True
        run_start[1:] = sk[1:] != sk[:-1]
        starts = np.flatnonzero(run_start)
        run_id = np.cumsum(run_start) - 1
        occ_sorted = np.arange(M) - starts[run_id]
        occ[e, order] = occ_sorted

    while True:
        is_main = occ < NMAIN
        mcount_try = np.zeros((E, NB * NB, NMAIN), dtype=np.int64)
        mch = (occ + tgt) % NMAIN
        for e in range(E):
            np.add.at(mcount_try[e], (gid[e][is_main[e]], mch[e][is_main[e]]), 1)
        if (-(-mcount_try.max(axis=0) // 128) * 128).max() <= CHUNK:
            break
        NMAIN += 1
    mchunk = (occ + tgt) % NMAIN                # main chunk within group
    olevel = occ - NMAIN                        # overflow level (>=0 where not main)
    n_ovf = int(olevel.max()) + 1 if (~is_main).any() else 0

    # per (core, group, main-chunk) counts -> harmonized caps
    mcount = np.zeros((E, NB * NB, NMAIN), dtype=np.int64)
    ocount = np.zeros((E, NB * NB, max(n_ovf, 1)), dtype=np.int64)
    for e in range(E):
        np.add.at(mcount[e], (gid[e][is_main[e]], mchunk[e][is_main[e]]), 1)
        if n_ovf:
            sel = ~is_main[e]
            np.add.at(ocount[e], (gid[e][sel], olevel[e][sel]), 1)
    mcap = -(-mcount.max(axis=0) // 128) * 128            # [G, NMAIN]
    ocap = -(-ocount.max(axis=0) // 128) * 128            # [G, n_ovf]

    # chunk list: interleave tb so consecutive scatters hit different windows
    chunks = []
    col = 0
    rounds = [("m", r) for r in range(NMAIN)] + [("o", r) for r in range(n_ovf)]
    chunk_off = {}                              # (g, kind, r) -> (off, size)
    for kind, r in rounds:
        for t in range(NB):
            for s_ in range(NB):
                g = s_ * NB + t
                size = int(mcap[g, r] if kind == "m" else ocap[g, r])
                if size == 0:
                    continue
                chunks.append((s_, t, size, col))
                chunk_off[(g, kind, r)] = (col, size)
                col += size
    tot = col
    assert tot % 128 == 0

    gsrc = np.zeros((E, tot), dtype=np.int16)
    gtgt = np.zeros((E, tot), dtype=np.int16)
    pad_t = (BS + (np.arange(tot) % JUNK)).astype(np.int16)
    gtgt[:] = pad_t[None, :]

    for e in range(E):
        for g in range(NB * NB):
            for kind, r in rounds:
                if (g, kind, r) not in chunk_off:
                    continue
                off, size = chunk_off[(g, kind, r)]
                if kind == "m":
                    sel = (gid[e] == g) & is_main[e] & (mchunk[e] == r)
                else:
                    sel = (gid[e] == g) & ~is_main[e] & (olevel[e] == r)
                n = int(sel.sum())
                assert n <= size
                gsrc[e, off:off + n] = (src[e, sel] % BS).astype(np.int16)
                gtgt[e, off:off + n] = (tgt[e, sel] % BS).astype(np.int16)

    # wrap [tot] -> [16, tot//16] (element i at (i % 16, i // 16)), then
    # replicate 8x across partition stripes (one copy per GPSIMD core)
    gsrc_w = np.tile(gsrc.reshape(E, -1, 16).transpose(0, 2, 1), (1, 8, 1))
    gtgt_w = np.tile(gtgt.reshape(E, -1, 16).transpose(0, 2, 1), (1, 8, 1))
    return chunks, tot, np.ascontiguousarray(gsrc_w), np.ascontiguousarray(gtgt_w)


def build_bass(chunks, tot):
    nc = bacc.Bacc("TRN2", target_bir_lowering=False)

    x_d = nc.dram_tensor("x", [N, D], F32, kind="ExternalInput")
    wt_d = nc.dram_tensor("wt", [D, D], F32, kind="ExternalInput")     # W_e^T
    bb_d = nc.dram_tensor("bb", [D, D], F32, kind="ExternalInput")     # b_e bcast
    gs_d = nc.dram_tensor("gsrc", [128, tot // 16], I16, kind="ExternalInput")
    gt_d = nc.dram_tensor("gtgt", [128, tot // 16], I16, kind="ExternalInput")
    proj_d = nc.dram_tensor("proj", [N, D], F32)                       # internal
    msg_d = nc.dram_tensor("msg", [NB * TBS, D], F32, kind="ExternalOutput")

    with tile.TileContext(nc) as tc:
        with (
            tc.tile_pool(name="const", bufs=1) as constp,
            tc.tile_pool(name="xin", bufs=3) as xp,
            tc.tile_pool(name="xt", bufs=4) as xtp,
            tc.tile_pool(name="pout", bufs=3) as op,
            tc.tile_pool(name="gat", bufs=4) as gp,
            tc.tile_pool(name="idx", bufs=4) as ip,
            tc.tile_pool(name="ps1", bufs=4, space="PSUM") as ps1,
            tc.tile_pool(name="ps2", bufs=4, space="PSUM") as ps2,
        ):
            ident = constp.tile([128, 128], F32)
            make_identity(nc, ident[:])
            wt_s = constp.tile([D, D], F32)
            nc.sync.dma_start(wt_s[:], wt_d[:])
            bb_s = constp.tile([D, D], F32)
            nc.sync.dma_start(bb_s[:], bb_d[:])

            # ---- Phase A: proj = x @ W^T + b, 1024-node supertiles ----
            for n0 in range(0, N, SUPER):
                ns = min(SUPER, N - n0)
                nblk = -(-ns // 128)
                full = ns // 128
                rem = ns % 128
                xb = xp.tile([128, nblk, D], F32, tag="xin")
                # x[n0:n0+ns] viewed as [128, nblk, D] (node = n0 + c*128 + p)
                if full:
                    nc.sync.dma_start(
                        xb[:, :full, :],
                        x_d[n0:n0 + full * 128, :].rearrange(
                            "(c p) d -> p c d", p=128
                        ),
                    )
                if rem:
                    # ragged tail rows loaded separately into last block
                    nc.sync.dma_start(
                        xb[:rem, full, :],
                        x_d[n0 + full * 128:n0 + ns, :],
                    )
                ob = op.tile([128, nblk, D], F32, tag="pout")
                for c in range(nblk):
                    bp = 128 if (c + 1) * 128 <= ns else ns - c * 128
                    p1 = ps1.tile([128, 128], F32, tag="ps1")
                    nc.tensor.transpose(p1[:, :bp], xb[:bp, c, :], ident[:bp, :bp])
                    xt = xtp.tile([128, 128], F32, tag="xt")
                    nc.vector.tensor_copy(xt[:, :bp], p1[:, :bp])
                    p2 = ps2.tile([128, D], F32, tag="ps2")
                    nc.tensor.matmul(p2[:bp, :], xt[:, :bp], wt_s[:])
                    nc.vector.tensor_add(ob[:bp, c, :], p2[:bp, :], bb_s[:bp, :])
                if full:
                    nc.sync.dma_start(
                        proj_d[n0:n0 + full * 128, :].rearrange(
                            "(c p) d -> p c d", p=128
                        ),
                        ob[:, :full, :],
                    )
                if rem:
                    nc.sync.dma_start(
                        proj_d[n0 + full * 128:n0 + ns, :], ob[:rem, full, :]
                    )

            # ---- Phase B/C: gather from proj by src, scatter-add to msg ----
            for (sbk, tbk, size, off) in chunks:
                si = ip.tile([128, size // 16], I16, tag="sidx")
                nc.sync.dma_start(si[:], gs_d[:, off // 16:(off + size) // 16])
                ti = ip.tile([128, size // 16], I16, tag="tidx")
                nc.sync.dma_start(ti[:], gt_d[:, off // 16:(off + size) // 16])
                g = gp.tile([128, size // 128, D], F32, tag="gat")
                nc.gpsimd.dma_gather(
                    g[:],
                    proj_d[sbk * BS:(sbk + 1) * BS, :],
                    si[:],
                    size,
                    size,
                    D,
                    queue_num=0,
                )
                nc.gpsimd.dma_scatter_add(
                    msg_d[tbk * TBS:tbk * TBS + TBS, :],
                    g[:],
                    ti[:],
                    size,
                    size,
                    D,
                    queue_num=0,
                )

    nc.compile()
    return nc


def kernel(edge_lists, node_states, W, b):
    edge_lists = np.asarray(edge_lists)
    node_states = np.asarray(node_states, dtype=np.float32)
    W = np.asarray(W, dtype=np.float32)
    b = np.asarray(b, dtype=np.float32)

    chunks, tot, gsrc_w, gtgt_w = build_schedule(edge_lists)
    nc = build_bass(chunks, tot)

    in_maps = []
    for e in range(E):
        we_t = np.ascontiguousarray(W[e * D:(e + 1) * D, :].T)         # [k, j]
        bb = np.ascontiguousarray(
            np.broadcast_to(b[e * D:(e + 1) * D], (D, D))
        )
        in_maps.append(
            {
                "x": node_states,
                "wt": we_t,
                "bb": bb,
                "gsrc": gsrc_w[e],
                "gtgt": gtgt_w[e],
            }
        )

    global LAST
    res = run_bass_kernel_spmd(nc, in_maps, core_ids=list(range(E)), trace=TRACE)
    LAST = res

    total = np.zeros((N, D), dtype=np.float32)
    for e in range(E):
        buf = res.results[e]["msg"]
        for bkt in range(NB):
            total[bkt * BS:(bkt + 1) * BS] += buf[bkt * TBS:bkt * TBS + BS]

    counts = np.bincount(
        np.asarray(edge_lists[:, :, 1]).reshape(-1), minlength=N
    ).astype(np.float32)
    divisor = np.where(counts == 0.0, 1.0, counts)
    return total / divisor[:, None]



# revision 2
# speedup vs baseline: 1.0562x; 1.0562x over previous
"""GNN message-passing on 8 Trainium2 cores — gather + segment-sum matmul.

Per core e (edge-type sharding):
  phase A: proj_e = x @ W_e^T   (PE fp16 in, f32 PSUM, fp16 out) -> DRAM
  phase B: edges grouped (src_bucket, tgt_window); dma_gather proj rows
           (1024/call, the HW limit); aggregate per 128-target window with
           one-hot selection matmuls on the PE:
             S_strip[p, :] = (iota == goff[p, col])      (DVE is_equal, fp16)
             psum[w] += S_strip[:, k*128:...]^T @ gathered_col
           flush each (bucket-run, window) PSUM tile -> fp16 -> DRAM.
  host:    sum bucket partials per window, add bias*counts_e, divide by
           global in-degree. (Cross-core reduce on host, as the baseline.)

The pass/flush schedule is SPMD-static: per-(bucket, window) slot segments
sized cap = max over cores; padding slots gather row 0 with off=PADOFF so
the one-hot never matches (contributes exactly 0).
"""

import numpy as np

import concourse.bacc as bacc
import concourse.mybir as mybir
import concourse.tile as tile
from concourse.bass_utils import run_bass_kernel_spmd

N = 100000    # nodes
D = 128       # hidden
E = 8         # edge types == cores
M = 200000    # edges per type

TW = 128      # targets per window
CALL = 1024   # max idxs per dma_gather (HW limit)
KMAX = 5      # max windows a 128-slot column may straddle
PADOFF = 2000.0

NPAD = -(-N // 512) * 512
BS = NPAD // 4
NB = 4
NWIN = NPAD // TW
STRIP = KMAX * TW

F32 = mybir.dt.float32
F16 = mybir.dt.float16
I16 = mybir.dt.int16

TRACE = False
LAST = None


def _derive(n, m):
    global N, M, NPAD, BS, NB, NWIN, STRIP
    N, M = n, m
    NPAD = -(-N // 512) * 512
    BS = NPAD // 4
    NWIN = NPAD // TW
    STRIP = KMAX * TW


def build_schedule(edge_lists):
    """Static layout shared by all cores + per-core index/offset data."""
    src = np.asarray(edge_lists[:, :, 0], dtype=np.int64)
    tgt = np.asarray(edge_lists[:, :, 1], dtype=np.int64)
    sb = src // BS
    wq = tgt // TW

    counts = np.zeros((E, NB, NWIN), dtype=np.int64)
    for e in range(E):
        np.add.at(counts[e], (sb[e], wq[e]), 1)
    caps = counts.max(axis=0)                       # [NB, NWIN]

    # slot layout: runs by sb, segments by window; run length padded to x128
    seg_start = np.zeros((NB, NWIN), dtype=np.int64)
    run_start = np.zeros(NB + 1, dtype=np.int64)
    pos = 0
    for b in range(NB):
        run_start[b] = pos
        for wi in range(NWIN):
            seg_start[b, wi] = pos
            pos += caps[b, wi]
        pos = -(-pos // 128) * 128
    run_start[NB] = pos
    tot = pos
    ncol = tot // 128

    seg_flat_start = seg_start.reshape(-1)
    seg_flat_end = seg_flat_start + caps.reshape(-1)

    # per-column static meta: (bucket, base window, K windows)
    col_meta = []
    for c in range(ncol):
        lo, hi = c * 128, c * 128 + 128
        b = int(np.searchsorted(run_start[1:], lo, side="right"))
        s = seg_flat_start[b * NWIN:(b + 1) * NWIN]
        t = seg_flat_end[b * NWIN:(b + 1) * NWIN]
        wins = np.flatnonzero((s < hi) & (t > lo))
        if len(wins) == 0:
            col_meta.append((b, 0, 0))
            continue
        wb, wl = int(wins[0]), int(wins[-1])
        K = wl - wb + 1
        assert K <= KMAX, f"col {c}: K={K} > KMAX"
        col_meta.append((b, wb, K))

    # first/last column touching each (b, w); bidx = dense flush index
    first_col, last_col = {}, {}
    for c, (b, wb, K) in enumerate(col_meta):
        for k in range(K):
            key = (b, wb + k)
            first_col.setdefault(key, c)
            last_col[key] = c
    # bidx assigned in FLUSH order (program order of last_col events) so
    # flushed tiles can be staged and DMA'd out in contiguous batches
    flush_events = []                               # (col, k) -> key
    for c, (b, wb, K) in enumerate(col_meta):
        for k in range(K):
            key = (b, wb + k)
            if last_col[key] == c:
                flush_events.append(key)
    bidx_map = {key: i for i, key in enumerate(flush_events)}
    nflush = len(bidx_map)

    # per-core slot data
    wb_per_col = np.array([m_[1] for m_ in col_meta], dtype=np.int64)
    gsrc = np.zeros((E, tot), dtype=np.int16)
    goff = np.full((E, 128, ncol), PADOFF, dtype=np.float32)
    for e in range(E):
        order = np.lexsort((tgt[e], wq[e], sb[e]))
        se, te = src[e][order], tgt[e][order]
        key = sb[e][order] * NWIN + wq[e][order]
        grp_first = np.searchsorted(key, key, side="left")
        rank = np.arange(M) - grp_first
        slot = seg_flat_start[key] + rank
        gsrc[e, slot] = (se % BS).astype(np.int16)
        col = slot // 128
        p = slot % 128
        goff[e, p, col] = (te - wb_per_col[col] * TW).astype(np.float32)
    assert goff[goff != PADOFF].max(initial=0) < STRIP

    gsrc_w = np.ascontiguousarray(
        np.tile(gsrc.reshape(E, -1, 16).transpose(0, 2, 1), (1, 8, 1))
    )
    return dict(
        caps=caps, gsrc_w=gsrc_w, goff=goff.astype(np.float16),
        col_meta=col_meta, first_col=first_col, last_col=last_col,
        bidx_map=bidx_map, tot=tot, nflush=nflush, run_start=run_start,
    )


def build_bass(sched):
    col_meta = sched["col_meta"]
    first_col, last_col = sched["first_col"], sched["last_col"]
    bidx_map = sched["bidx_map"]
    tot, nflush = sched["tot"], sched["nflush"]
    run_start = sched["run_start"]
    ncol = tot // 128

    nc = bacc.Bacc("TRN2", target_bir_lowering=False)

    xt_d = nc.dram_tensor("xt", [D, NPAD], F16, kind="ExternalInput")
    wt_d = nc.dram_tensor("wt", [D, D], F16, kind="ExternalInput")
    gs_d = nc.dram_tensor("gsrc", [128, tot // 16], I16, kind="ExternalInput")
    go_d = nc.dram_tensor("goff", [128, ncol], F16, kind="ExternalInput")
    iota_d = nc.dram_tensor("iota", [128, STRIP], F16, kind="ExternalInput")
    proj_d = nc.dram_tensor("proj", [NPAD, D], F16)
    msg_d = nc.dram_tensor("msg", [nflush * 128, D], F16, kind="ExternalOutput")

    with tile.TileContext(nc) as tc:
        with (
            tc.tile_pool(name="const", bufs=1) as constp,
            tc.tile_pool(name="xtp", bufs=3) as xtp,
            tc.tile_pool(name="pout", bufs=3) as pop,
            tc.tile_pool(name="psA", bufs=2, space="PSUM") as psA,
            tc.tile_pool(name="gat", bufs=6) as gp,
            tc.tile_pool(name="idx", bufs=6) as ip,
            tc.tile_pool(name="strip", bufs=6) as sp,
            tc.tile_pool(name="psB", bufs=6, space="PSUM") as psB,
            tc.tile_pool(name="fl", bufs=6) as fp,
        ):
            wt_s = constp.tile([D, D], F16)
            nc.sync.dma_start(wt_s[:], wt_d[:])
            iota_s = constp.tile([128, STRIP], F16)
            nc.sync.dma_start(iota_s[:], iota_d[:])
            goff_s = constp.tile([128, ncol], F16)
            nc.sync.dma_start(goff_s[:], go_d[:])

            # ---- Phase A: proj = x @ W^T (fp16) ----
            XTCH = 4096
            for n0 in range(0, NPAD, XTCH):
                nch = min(XTCH, NPAD - n0)
                xt_t = xtp.tile([128, XTCH], F16, tag="xt")
                nc.sync.dma_start(xt_t[:, :nch], xt_d[:, n0:n0 + nch])
                ob = pop.tile([128, XTCH // 128, D], F16, tag="pout")
                for ci in range(nch // 128):
                    pA = psA.tile([128, D], F32, tag="psA")
                    nc.tensor.matmul(
                        pA[:], xt_t[:, ci * 128:(ci + 1) * 128], wt_s[:],
                        start=True, stop=True,
                    )
                    nc.scalar.activation(ob[:, ci, :], pA[:], mybir.ActivationFunctionType.Copy)
                nc.sync.dma_start(
                    proj_d[n0:n0 + nch, :].rearrange("(c p) d -> p c d", p=128),
                    ob[:, : nch // 128, :],
                )

            # ---- Phase B: gather + segment matmuls ----
            FB = 8
            psum_tiles = {}
            stage = None
            for b in range(NB):
                lo, hi = int(run_start[b]), int(run_start[b + 1])
                for off in range(lo, hi, CALL):
                    sz = min(CALL, hi - off)
                    si = ip.tile([128, CALL // 16], I16, tag="sidx")
                    nc.sync.dma_start(
                        si[:, : sz // 16], gs_d[:, off // 16:(off + sz) // 16]
                    )
                    g = gp.tile([128, CALL // 128, D], F16, tag="gat")
                    nc.gpsimd.dma_gather(
                        g[:, : sz // 128, :], proj_d[b * BS:(b + 1) * BS, :],
                        si[:, : sz // 16], sz, sz, D, queue_num=0,
                    )
                    for cc in range(off // 128, (off + sz) // 128):
                        bb, wb, K = col_meta[cc]
                        if K == 0:
                            continue
                        gci = cc - off // 128
                        strip = sp.tile([128, STRIP], F16, tag="strip")
                        nc.vector.tensor_tensor(
                            strip[:, : K * TW],
                            iota_s[:, : K * TW],
                            goff_s[:, cc:cc + 1].broadcast_to([128, K * TW]),
                            mybir.AluOpType.is_equal,
                        )
                        for k in range(K):
                            key = (bb, wb + k)
                            if first_col[key] == cc:
                                pt = psB.tile([128, D], F32, tag="psB")
                                psum_tiles[key] = pt
                            else:
                                pt = psum_tiles[key]
                            is_last = last_col[key] == cc
                            nc.tensor.matmul(
                                pt[:], strip[:, k * TW:(k + 1) * TW],
                                g[:, gci, :],
                                start=(first_col[key] == cc), stop=is_last,
                            )
                            if is_last:
                                pt = psum_tiles.pop(key)
                                bidx = bidx_map[key]
                                sl = bidx % FB
                                if sl == 0:
                                    stage = fp.tile([128, FB, D], F16, tag="fl")
                                nc.scalar.activation(
                                    stage[:, sl, :], pt[:],
                                    mybir.ActivationFunctionType.Copy,
                                )
                                if sl == FB - 1 or bidx == nflush - 1:
                                    b0 = bidx - sl
                                    nc.sync.dma_start(
                                        msg_d[b0 * 128:(b0 + sl + 1) * 128, :]
                                        .rearrange("(c p) d -> p c d", p=128),
                                        stage[:, : sl + 1, :],
                                    )
    nc.compile()
    return nc


def kernel(edge_lists, node_states, W=None, b=None, **kw):
    global LAST
    W_in = W if W is not None else kw["W"]
    b_in = b if b is not None else kw["b"]
    edge_lists = np.asarray(edge_lists)
    x = np.asarray(node_states, dtype=np.float32)
    Wm = np.asarray(W_in, dtype=np.float32)
    bv = np.asarray(b_in, dtype=np.float32)

    sched = build_schedule(edge_lists)
    nc = build_bass(sched)

    xt = np.zeros((D, NPAD), dtype=np.float32)
    xt[:, :N] = x.T
    xt = xt.astype(np.float16)
    iota = np.ascontiguousarray(
        np.broadcast_to(np.arange(STRIP, dtype=np.float16), (128, STRIP))
    )

    in_maps = []
    for e in range(E):
        wt = np.ascontiguousarray(Wm[e * D:(e + 1) * D, :].T).astype(np.float16)
        in_maps.append(
            {
                "xt": xt,
                "wt": wt,
                "gsrc": sched["gsrc_w"][e],
                "goff": np.ascontiguousarray(sched["goff"][e]),
                "iota": iota,
            }
        )

    res = run_bass_kernel_spmd(
        nc, in_maps, core_ids=list(range(E)), trace=TRACE
    )
    LAST = res

    tgt_all = np.asarray(edge_lists[:, :, 1], dtype=np.int64)
    total = np.zeros((NPAD, D), dtype=np.float32)
    bidx_map = sched["bidx_map"]
    for e in range(E):
        msg = np.asarray(res.results[e]["msg"]).astype(np.float32)
        for (b_, wi), bidx in bidx_map.items():
            total[wi * TW:(wi + 1) * TW] += msg[bidx * 128:(bidx + 1) * 128]
        ce = np.bincount(tgt_all[e], minlength=NPAD).astype(np.float32)
        total += ce[:, None] * bv[e * D:(e + 1) * D][None, :]

    counts = np.bincount(tgt_all.reshape(-1), minlength=NPAD).astype(np.float32)
    divisor = np.where(counts == 0.0, 1.0, counts)
    return (total / divisor[:, None])[:N].astype(np.float32)


# revision 3
# speedup vs baseline: 1.0843x; 1.0266x over previous
"""GNN message-passing on 8 Trainium2 cores — gather + segment-sum matmul.

Per core e (edge-type sharding):
  phase A: proj_e = x @ W_e^T   (PE fp16 in, f32 PSUM, fp16 out) -> DRAM
  phase B: edges grouped (src_bucket, tgt_window); dma_gather proj rows
           (1024/call, the HW limit); aggregate per 128-target window with
           one-hot selection matmuls on the PE:
             S_strip[p, :] = (iota == goff[p, col])      (DVE is_equal, fp16)
             psum[w] += S_strip[:, k*128:...]^T @ gathered_col
           flush each (bucket-run, window) PSUM tile -> fp16 -> DRAM.
  host:    sum bucket partials per window, add bias*counts_e, divide by
           global in-degree. (Cross-core reduce on host, as the baseline.)

The pass/flush schedule is SPMD-static: per-(bucket, window) slot segments
sized cap = max over cores; padding slots gather row 0 with off=PADOFF so
the one-hot never matches (contributes exactly 0).
"""

import numpy as np

import concourse.bacc as bacc
import concourse.mybir as mybir
import concourse.tile as tile
from concourse.bass_utils import run_bass_kernel_spmd

N = 100000    # nodes
D = 128       # hidden
E = 8         # edge types == cores
M = 200000    # edges per type

TW = 128      # targets per window
BL = 2        # windows per harmonization block (256 targets)
CALL = 1024   # max idxs per dma_gather (HW limit)
KMAX = 6      # max windows a 128-slot column may straddle
PADOFF = 2000.0

NPAD = -(-N // 512) * 512
BS = NPAD // 4
NB = 4
NWIN = NPAD // TW
STRIP = KMAX * TW

F32 = mybir.dt.float32
F16 = mybir.dt.float16
I16 = mybir.dt.int16

TRACE = False
LAST = None


def _derive(n, m):
    global N, M, NPAD, BS, NB, NWIN, STRIP
    N, M = n, m
    NPAD = -(-N // 512) * 512
    BS = NPAD // 4
    NWIN = NPAD // TW
    STRIP = KMAX * TW


def build_schedule(edge_lists):
    """Static layout shared by all cores + per-core index/offset data."""
    src = np.asarray(edge_lists[:, :, 0], dtype=np.int64)
    tgt = np.asarray(edge_lists[:, :, 1], dtype=np.int64)
    sb = src // BS
    wq = tgt // TW

    NBLK = -(-NWIN // BL)                           # harmonization blocks
    blk = wq // BL
    counts = np.zeros((E, NB, NBLK), dtype=np.int64)
    for e in range(E):
        np.add.at(counts[e], (sb[e], blk[e]), 1)
    caps = counts.max(axis=0)                       # [NB, NBLK]

    # slot layout: runs by sb, segments by block; run length padded to x128
    seg_start = np.zeros((NB, NBLK), dtype=np.int64)
    run_start = np.zeros(NB + 1, dtype=np.int64)
    pos = 0
    for b in range(NB):
        run_start[b] = pos
        for bi in range(NBLK):
            seg_start[b, bi] = pos
            pos += caps[b, bi]
        pos = -(-pos // 128) * 128
    run_start[NB] = pos
    tot = pos
    ncol = tot // 128

    seg_flat_start = seg_start.reshape(-1)
    seg_flat_end = seg_flat_start + caps.reshape(-1)

    # per-column static meta: (bucket, base window, K windows)
    col_meta = []
    for c in range(ncol):
        lo, hi = c * 128, c * 128 + 128
        b = int(np.searchsorted(run_start[1:], lo, side="right"))
        s = seg_flat_start[b * NBLK:(b + 1) * NBLK]
        t = seg_flat_end[b * NBLK:(b + 1) * NBLK]
        blks = np.flatnonzero((s < hi) & (t > lo))
        if len(blks) == 0:
            col_meta.append((b, 0, 0))
            continue
        wb = int(blks[0]) * BL
        wl = min(int(blks[-1]) * BL + BL - 1, NWIN - 1)
        K = wl - wb + 1
        assert K <= KMAX, f"col {c}: K={K} > KMAX"
        col_meta.append((b, wb, K))

    # first/last column touching each (b, w); bidx = dense flush index
    first_col, last_col = {}, {}
    for c, (b, wb, K) in enumerate(col_meta):
        for k in range(K):
            key = (b, wb + k)
            first_col.setdefault(key, c)
            last_col[key] = c
    # bidx assigned in FLUSH order (program order of last_col events) so
    # flushed tiles can be staged and DMA'd out in contiguous batches
    flush_events = []                               # (col, k) -> key
    for c, (b, wb, K) in enumerate(col_meta):
        for k in range(K):
            key = (b, wb + k)
            if last_col[key] == c:
                flush_events.append(key)
    bidx_map = {key: i for i, key in enumerate(flush_events)}
    nflush = len(bidx_map)

    # per-core slot data
    wb_per_col = np.array([m_[1] for m_ in col_meta], dtype=np.int64)
    gsrc = np.zeros((E, tot), dtype=np.int16)
    goff = np.full((E, 128, ncol), PADOFF, dtype=np.float32)
    for e in range(E):
        order = np.lexsort((tgt[e], blk[e], sb[e]))
        se, te = src[e][order], tgt[e][order]
        key = sb[e][order] * NBLK + blk[e][order]
        grp_first = np.searchsorted(key, key, side="left")
        rank = np.arange(M) - grp_first
        slot = seg_flat_start[key] + rank
        gsrc[e, slot] = (se % BS).astype(np.int16)
        col = slot // 128
        p = slot % 128
        goff[e, p, col] = (te - wb_per_col[col] * TW).astype(np.float32)
    assert goff[goff != PADOFF].max(initial=0) < STRIP

    gsrc_w = np.ascontiguousarray(
        np.tile(gsrc.reshape(E, -1, 16).transpose(0, 2, 1), (1, 8, 1))
    )
    return dict(
        caps=caps, gsrc_w=gsrc_w, goff=goff.astype(np.float16),
        col_meta=col_meta, first_col=first_col, last_col=last_col,
        bidx_map=bidx_map, tot=tot, nflush=nflush, run_start=run_start,
    )


def build_bass(sched):
    col_meta = sched["col_meta"]
    first_col, last_col = sched["first_col"], sched["last_col"]
    bidx_map = sched["bidx_map"]
    tot, nflush = sched["tot"], sched["nflush"]
    run_start = sched["run_start"]
    ncol = tot // 128

    nc = bacc.Bacc("TRN2", target_bir_lowering=False)

    xt_d = nc.dram_tensor("xt", [D, NPAD], F16, kind="ExternalInput")
    wt_d = nc.dram_tensor("wt", [D, D], F16, kind="ExternalInput")
    gs_d = nc.dram_tensor("gsrc", [128, tot // 16], I16, kind="ExternalInput")
    go_d = nc.dram_tensor("goff", [128, ncol], F16, kind="ExternalInput")
    iota_d = nc.dram_tensor("iota", [128, STRIP], F16, kind="ExternalInput")
    proj_d = nc.dram_tensor("proj", [NPAD, D], F16)
    msg_d = nc.dram_tensor("msg", [nflush * 128, D], F16, kind="ExternalOutput")

    with tile.TileContext(nc) as tc:
        with (
            tc.tile_pool(name="const", bufs=1) as constp,
            tc.tile_pool(name="xtp", bufs=3) as xtp,
            tc.tile_pool(name="pout", bufs=3) as pop,
            tc.tile_pool(name="psA", bufs=2, space="PSUM") as psA,
            tc.tile_pool(name="gat", bufs=6) as gp,
            tc.tile_pool(name="strip", bufs=6) as sp,
            tc.tile_pool(name="psB", bufs=6, space="PSUM") as psB,
            tc.tile_pool(name="fl", bufs=6) as fp,
        ):
            wt_s = constp.tile([D, D], F16)
            nc.sync.dma_start(wt_s[:], wt_d[:])
            iota_s = constp.tile([128, STRIP], F16)
            nc.sync.dma_start(iota_s[:], iota_d[:])
            goff_s = constp.tile([128, ncol], F16)
            nc.sync.dma_start(goff_s[:], go_d[:])
            gs_s = constp.tile([128, tot // 16], I16)
            nc.sync.dma_start(gs_s[:], gs_d[:])

            # ---- Phase A: proj = x @ W^T (fp16) ----
            # 4 node-chunks per PSUM bank; one wide PSUM->SBUF copy per bank,
            # alternating Act/DVE so bucket 0 is ready for gathers ASAP.
            XTCH = 4096
            for n0 in range(0, NPAD, XTCH):
                nch = min(XTCH, NPAD - n0)
                xt_t = xtp.tile([128, XTCH], F16, tag="xt")
                nc.sync.dma_start(xt_t[:, :nch], xt_d[:, n0:n0 + nch])
                ob = pop.tile([128, XTCH // 128, D], F16, tag="pout")
                for c4 in range(0, nch // 128, 4):
                    nb4 = min(4, nch // 128 - c4)
                    pA = psA.tile([128, 4, D], F32, tag="psA")
                    for ci in range(c4, c4 + nb4):
                        nc.tensor.matmul(
                            pA[:, ci - c4, :],
                            xt_t[:, ci * 128:(ci + 1) * 128], wt_s[:],
                            start=True, stop=True,
                        )
                    eng = nc.scalar if (c4 // 4) % 2 == 0 else nc.vector
                    if eng is nc.scalar:
                        nc.scalar.activation(
                            ob[:, c4:c4 + nb4, :], pA[:, :nb4, :],
                            mybir.ActivationFunctionType.Copy,
                        )
                    else:
                        nc.vector.tensor_copy(
                            ob[:, c4:c4 + nb4, :], pA[:, :nb4, :]
                        )
                nc.sync.dma_start(
                    proj_d[n0:n0 + nch, :].rearrange("(c p) d -> p c d", p=128),
                    ob[:, : nch // 128, :],
                )

            # ---- Phase B: gather + segment matmuls ----
            FB = 8
            psum_tiles = {}
            stage = None
            for b in range(NB):
                lo, hi = int(run_start[b]), int(run_start[b + 1])
                for off in range(lo, hi, CALL):
                    sz = min(CALL, hi - off)
                    g = gp.tile([128, CALL // 128, D], F16, tag="gat")
                    nc.gpsimd.dma_gather(
                        g[:, : sz // 128, :], proj_d[b * BS:(b + 1) * BS, :],
                        gs_s[:, off // 16:(off + sz) // 16], sz, sz, D,
                        queue_num=0,
                    )
                    for cc in range(off // 128, (off + sz) // 128):
                        bb, wb, K = col_meta[cc]
                        if K == 0:
                            continue
                        gci = cc - off // 128
                        strip = sp.tile([128, STRIP], F16, tag="strip")
                        nc.vector.tensor_tensor(
                            strip[:, : K * TW],
                            iota_s[:, : K * TW],
                            goff_s[:, cc:cc + 1].broadcast_to([128, K * TW]),
                            mybir.AluOpType.is_equal,
                        )
                        for k in range(K):
                            key = (bb, wb + k)
                            if first_col[key] == cc:
                                pt = psB.tile([128, D], F32, tag="psB")
                                psum_tiles[key] = pt
                            else:
                                pt = psum_tiles[key]
                            is_last = last_col[key] == cc
                            nc.tensor.matmul(
                                pt[:], strip[:, k * TW:(k + 1) * TW],
                                g[:, gci, :],
                                start=(first_col[key] == cc), stop=is_last,
                            )
                            if is_last:
                                pt = psum_tiles.pop(key)
                                bidx = bidx_map[key]
                                sl = bidx % FB
                                if sl == 0:
                                    stage = fp.tile([128, FB, D], F16, tag="fl")
                                nc.scalar.activation(
                                    stage[:, sl, :], pt[:],
                                    mybir.ActivationFunctionType.Copy,
                                )
                                if sl == FB - 1 or bidx == nflush - 1:
                                    b0 = bidx - sl
                                    nc.sync.dma_start(
                                        msg_d[b0 * 128:(b0 + sl + 1) * 128, :]
                                        .rearrange("(c p) d -> p c d", p=128),
                                        stage[:, : sl + 1, :],
                                    )
    nc.compile()
    return nc


def kernel(edge_lists, node_states, W=None, b=None, **kw):
    global LAST
    W_in = W if W is not None else kw["W"]
    b_in = b if b is not None else kw["b"]
    edge_lists = np.asarray(edge_lists)
    x = np.asarray(node_states, dtype=np.float32)
    Wm = np.asarray(W_in, dtype=np.float32)
    bv = np.asarray(b_in, dtype=np.float32)

    sched = build_schedule(edge_lists)
    nc = build_bass(sched)

    xt = np.zeros((D, NPAD), dtype=np.float32)
    xt[:, :N] = x.T
    xt = xt.astype(np.float16)
    iota = np.ascontiguousarray(
        np.broadcast_to(np.arange(STRIP, dtype=np.float16), (128, STRIP))
    )

    in_maps = []
    for e in range(E):
        wt = np.ascontiguousarray(Wm[e * D:(e + 1) * D, :].T).astype(np.float16)
        in_maps.append(
            {
                "xt": xt,
                "wt": wt,
                "gsrc": sched["gsrc_w"][e],
                "goff": np.ascontiguousarray(sched["goff"][e]),
                "iota": iota,
            }
        )

    res = run_bass_kernel_spmd(
        nc, in_maps, core_ids=list(range(E)), trace=TRACE
    )
    LAST = res

    tgt_all = np.asarray(edge_lists[:, :, 1], dtype=np.int64)
    total = np.zeros((NPAD, D), dtype=np.float32)
    bidx_map = sched["bidx_map"]
    for e in range(E):
        msg = np.asarray(res.results[e]["msg"]).astype(np.float32)
        for (b_, wi), bidx in bidx_map.items():
            total[wi * TW:(wi + 1) * TW] += msg[bidx * 128:(bidx + 1) * 128]
        ce = np.bincount(tgt_all[e], minlength=NPAD).astype(np.float32)
        total += ce[:, None] * bv[e * D:(e + 1) * D][None, :]

    counts = np.bincount(tgt_all.reshape(-1), minlength=NPAD).astype(np.float32)
    divisor = np.where(counts == 0.0, 1.0, counts)
    return (total / divisor[:, None])[:N].astype(np.float32)


# revision 4
# speedup vs baseline: 1.1007x; 1.0151x over previous
"""GNN message-passing on 8 Trainium2 cores — gather + segment-sum matmul.

Per core e (edge-type sharding):
  phase A: proj_e = x @ W_e^T   (PE fp16 in, f32 PSUM, fp16 out) -> DRAM
  phase B: edges grouped (src_bucket, tgt_window); dma_gather proj rows
           (1024/call, the HW limit); aggregate per 128-target window with
           one-hot selection matmuls on the PE:
             S_strip[p, :] = (iota == goff[p, col])      (DVE is_equal, fp16)
             psum[w] += S_strip[:, k*128:...]^T @ gathered_col
           flush each (bucket-run, window) PSUM tile -> fp16 -> DRAM.
  host:    sum bucket partials per window, add bias*counts_e, divide by
           global in-degree. (Cross-core reduce on host, as the baseline.)

The pass/flush schedule is SPMD-static: per-(bucket, window) slot segments
sized cap = max over cores; padding slots gather row 0 with off=PADOFF so
the one-hot never matches (contributes exactly 0).
"""

import numpy as np

import concourse.bacc as bacc
import concourse.mybir as mybir
import concourse.tile as tile
from concourse.bass_utils import run_bass_kernel_spmd

N = 100000    # nodes
D = 128       # hidden
E = 8         # edge types == cores
M = 200000    # edges per type

TW = 128      # targets per window
BL = 2        # windows per harmonization block (256 targets)
CALL = 1024   # max idxs per dma_gather (HW limit)
KMAX = 6      # max windows a 128-slot column may straddle
PADOFF = 2000.0

NPAD = -(-N // 512) * 512
BS = NPAD // 4
NB = 4
NWIN = NPAD // TW
STRIP = KMAX * TW

F32 = mybir.dt.float32
F16 = mybir.dt.float16
I16 = mybir.dt.int16

TRACE = False
LAST = None


def _derive(n, m):
    global N, M, NPAD, BS, NB, NWIN, STRIP
    N, M = n, m
    NPAD = -(-N // 512) * 512
    BS = NPAD // 4
    NWIN = NPAD // TW
    STRIP = KMAX * TW


def build_schedule(edge_lists):
    """Static layout shared by all cores + per-core index/offset data."""
    src = np.asarray(edge_lists[:, :, 0], dtype=np.int64)
    tgt = np.asarray(edge_lists[:, :, 1], dtype=np.int64)
    sb = src // BS
    wq = tgt // TW

    NBLK = -(-NWIN // BL)                           # harmonization blocks
    blk = wq // BL
    counts = np.zeros((E, NB, NBLK), dtype=np.int64)
    for e in range(E):
        np.add.at(counts[e], (sb[e], blk[e]), 1)
    caps = counts.max(axis=0)                       # [NB, NBLK]

    # slot layout: runs by sb, segments by block; run length padded to x128
    seg_start = np.zeros((NB, NBLK), dtype=np.int64)
    run_start = np.zeros(NB + 1, dtype=np.int64)
    pos = 0
    for b in range(NB):
        run_start[b] = pos
        for bi in range(NBLK):
            seg_start[b, bi] = pos
            pos += caps[b, bi]
        pos = -(-pos // 128) * 128
    run_start[NB] = pos
    tot = pos
    ncol = tot // 128

    seg_flat_start = seg_start.reshape(-1)
    seg_flat_end = seg_flat_start + caps.reshape(-1)

    # per-column static meta: (bucket, base window, K windows)
    col_meta = []
    for c in range(ncol):
        lo, hi = c * 128, c * 128 + 128
        b = int(np.searchsorted(run_start[1:], lo, side="right"))
        s = seg_flat_start[b * NBLK:(b + 1) * NBLK]
        t = seg_flat_end[b * NBLK:(b + 1) * NBLK]
        blks = np.flatnonzero((s < hi) & (t > lo))
        if len(blks) == 0:
            col_meta.append((b, 0, 0))
            continue
        wb = int(blks[0]) * BL
        wl = min(int(blks[-1]) * BL + BL - 1, NWIN - 1)
        K = wl - wb + 1
        assert K <= KMAX, f"col {c}: K={K} > KMAX"
        col_meta.append((b, wb, K))

    # first/last column touching each (b, w); bidx = dense flush index
    first_col, last_col = {}, {}
    for c, (b, wb, K) in enumerate(col_meta):
        for k in range(K):
            key = (b, wb + k)
            first_col.setdefault(key, c)
            last_col[key] = c
    # bidx assigned in FLUSH order (program order of last_col events) so
    # flushed tiles can be staged and DMA'd out in contiguous batches
    flush_events = []                               # (col, k) -> key
    for c, (b, wb, K) in enumerate(col_meta):
        for k in range(K):
            key = (b, wb + k)
            if last_col[key] == c:
                flush_events.append(key)
    bidx_map = {key: i for i, key in enumerate(flush_events)}
    nflush = len(bidx_map)

    # per-core slot data
    wb_per_col = np.array([m_[1] for m_ in col_meta], dtype=np.int64)
    gsrc = np.zeros((E, tot), dtype=np.int16)
    goff = np.full((E, 128, ncol), PADOFF, dtype=np.float32)
    for e in range(E):
        order = np.lexsort((tgt[e], blk[e], sb[e]))
        se, te = src[e][order], tgt[e][order]
        key = sb[e][order] * NBLK + blk[e][order]
        grp_first = np.searchsorted(key, key, side="left")
        rank = np.arange(M) - grp_first
        slot = seg_flat_start[key] + rank
        gsrc[e, slot] = (se % BS).astype(np.int16)
        col = slot // 128
        p = slot % 128
        goff[e, p, col] = (te - wb_per_col[col] * TW).astype(np.float32)
    assert goff[goff != PADOFF].max(initial=0) < STRIP

    gsrc_w = np.ascontiguousarray(
        np.tile(gsrc.reshape(E, -1, 16).transpose(0, 2, 1), (1, 8, 1))
    )
    return dict(
        caps=caps, gsrc_w=gsrc_w, goff=goff.astype(np.float16),
        col_meta=col_meta, first_col=first_col, last_col=last_col,
        bidx_map=bidx_map, tot=tot, nflush=nflush, run_start=run_start,
    )


def build_bass(sched):
    col_meta = sched["col_meta"]
    first_col, last_col = sched["first_col"], sched["last_col"]
    bidx_map = sched["bidx_map"]
    tot, nflush = sched["tot"], sched["nflush"]
    run_start = sched["run_start"]
    ncol = tot // 128

    nc = bacc.Bacc("TRN2", target_bir_lowering=False)

    xt_d = nc.dram_tensor("xt", [D, NPAD], F16, kind="ExternalInput")
    wt_d = nc.dram_tensor("wt", [D, D], F16, kind="ExternalInput")
    gs_d = nc.dram_tensor("gsrc", [128, tot // 16], I16, kind="ExternalInput")
    go_d = nc.dram_tensor("goff", [128, ncol], F16, kind="ExternalInput")
    iota_d = nc.dram_tensor("iota", [128, STRIP], F16, kind="ExternalInput")
    proj_d = nc.dram_tensor("proj", [NPAD, D], F16)
    msg_d = nc.dram_tensor("msg", [nflush * 128, D], F16, kind="ExternalOutput")

    with tile.TileContext(nc) as tc:
        with (
            tc.tile_pool(name="const", bufs=1) as constp,
            tc.tile_pool(name="xtp", bufs=3) as xtp,
            tc.tile_pool(name="pout", bufs=3) as pop,
            tc.tile_pool(name="psA", bufs=2, space="PSUM") as psA,
            tc.tile_pool(name="gat", bufs=6) as gp,
            tc.tile_pool(name="strip", bufs=6) as sp,
            tc.tile_pool(name="psB", bufs=6, space="PSUM") as psB,
            tc.tile_pool(name="fl", bufs=6) as fp,
        ):
            wt_s = constp.tile([D, D], F16)
            nc.sync.dma_start(wt_s[:], wt_d[:])
            iota_s = constp.tile([128, STRIP], F16)
            nc.sync.dma_start(iota_s[:], iota_d[:])
            goff_s = constp.tile([128, ncol], F16)
            nc.sync.dma_start(goff_s[:], go_d[:])
            gs_s = constp.tile([128, tot // 16], I16)
            nc.sync.dma_start(gs_s[:], gs_d[:])

            # ---- Phase A: proj = x @ W^T (fp16) ----
            # 4 node-chunks per PSUM bank; one wide PSUM->SBUF copy per bank,
            # alternating Act/DVE so bucket 0 is ready for gathers ASAP.
            XTCH = 4096
            for n0 in range(0, NPAD, XTCH):
                nch = min(XTCH, NPAD - n0)
                xt_t = xtp.tile([128, XTCH], F16, tag="xt")
                nc.sync.dma_start(xt_t[:, :nch], xt_d[:, n0:n0 + nch])
                ob = pop.tile([128, XTCH // 128, D], F16, tag="pout")
                for c4 in range(0, nch // 128, 4):
                    nb4 = min(4, nch // 128 - c4)
                    pA = psA.tile([128, 4, D], F32, tag="psA")
                    for ci in range(c4, c4 + nb4):
                        nc.tensor.matmul(
                            pA[:, ci - c4, :],
                            xt_t[:, ci * 128:(ci + 1) * 128], wt_s[:],
                            start=True, stop=True,
                        )
                    eng = nc.scalar if (c4 // 4) % 2 == 0 else nc.vector
                    if eng is nc.scalar:
                        nc.scalar.activation(
                            ob[:, c4:c4 + nb4, :], pA[:, :nb4, :],
                            mybir.ActivationFunctionType.Copy,
                        )
                    else:
                        nc.vector.tensor_copy(
                            ob[:, c4:c4 + nb4, :], pA[:, :nb4, :]
                        )
                nc.sync.dma_start(
                    proj_d[n0:n0 + nch, :].rearrange("(c p) d -> p c d", p=128),
                    ob[:, : nch // 128, :],
                )

            # ---- Phase B: gather + segment matmuls ----
            FB = 8
            psum_tiles = {}
            stage = None
            for b in range(NB):
                lo, hi = int(run_start[b]), int(run_start[b + 1])
                for off in range(lo, hi, CALL):
                    sz = min(CALL, hi - off)
                    g = gp.tile([128, CALL // 128, D], F16, tag="gat")
                    nc.gpsimd.dma_gather(
                        g[:, : sz // 128, :], proj_d[b * BS:(b + 1) * BS, :],
                        gs_s[:, off // 16:(off + sz) // 16], sz, sz, D,
                        queue_num=0, single_packet=False,
                    )
                    for cc in range(off // 128, (off + sz) // 128):
                        bb, wb, K = col_meta[cc]
                        if K == 0:
                            continue
                        gci = cc - off // 128
                        strip = sp.tile([128, STRIP], F16, tag="strip")
                        nc.vector.tensor_tensor(
                            strip[:, : K * TW],
                            iota_s[:, : K * TW],
                            goff_s[:, cc:cc + 1].broadcast_to([128, K * TW]),
                            mybir.AluOpType.is_equal,
                        )
                        for k in range(K):
                            key = (bb, wb + k)
                            if first_col[key] == cc:
                                pt = psB.tile([128, D], F32, tag="psB")
                                psum_tiles[key] = pt
                            else:
                                pt = psum_tiles[key]
                            is_last = last_col[key] == cc
                            nc.tensor.matmul(
                                pt[:], strip[:, k * TW:(k + 1) * TW],
                                g[:, gci, :],
                                start=(first_col[key] == cc), stop=is_last,
                            )
                            if is_last:
                                pt = psum_tiles.pop(key)
                                bidx = bidx_map[key]
                                sl = bidx % FB
                                if sl == 0:
                                    stage = fp.tile([128, FB, D], F16, tag="fl")
                                nc.scalar.activation(
                                    stage[:, sl, :], pt[:],
                                    mybir.ActivationFunctionType.Copy,
                                )
                                if sl == FB - 1 or bidx == nflush - 1:
                                    b0 = bidx - sl
                                    nc.sync.dma_start(
                                        msg_d[b0 * 128:(b0 + sl + 1) * 128, :]
                                        .rearrange("(c p) d -> p c d", p=128),
                                        stage[:, : sl + 1, :],
                                    )
    nc.compile()
    return nc


def kernel(edge_lists, node_states, W=None, b=None, **kw):
    global LAST
    W_in = W if W is not None else kw["W"]
    b_in = b if b is not None else kw["b"]
    edge_lists = np.asarray(edge_lists)
    x = np.asarray(node_states, dtype=np.float32)
    Wm = np.asarray(W_in, dtype=np.float32)
    bv = np.asarray(b_in, dtype=np.float32)

    sched = build_schedule(edge_lists)
    nc = build_bass(sched)

    xt = np.zeros((D, NPAD), dtype=np.float32)
    xt[:, :N] = x.T
    xt = xt.astype(np.float16)
    iota = np.ascontiguousarray(
        np.broadcast_to(np.arange(STRIP, dtype=np.float16), (128, STRIP))
    )

    in_maps = []
    for e in range(E):
        wt = np.ascontiguousarray(Wm[e * D:(e + 1) * D, :].T).astype(np.float16)
        in_maps.append(
            {
                "xt": xt,
                "wt": wt,
                "gsrc": sched["gsrc_w"][e],
                "goff": np.ascontiguousarray(sched["goff"][e]),
                "iota": iota,
            }
        )

    res = run_bass_kernel_spmd(
        nc, in_maps, core_ids=list(range(E)), trace=TRACE
    )
    LAST = res

    tgt_all = np.asarray(edge_lists[:, :, 1], dtype=np.int64)
    total = np.zeros((NPAD, D), dtype=np.float32)
    bidx_map = sched["bidx_map"]
    for e in range(E):
        msg = np.asarray(res.results[e]["msg"]).astype(np.float32)
        for (b_, wi), bidx in bidx_map.items():
            total[wi * TW:(wi + 1) * TW] += msg[bidx * 128:(bidx + 1) * 128]
        ce = np.bincount(tgt_all[e], minlength=NPAD).astype(np.float32)
        total += ce[:, None] * bv[e * D:(e + 1) * D][None, :]

    counts = np.bincount(tgt_all.reshape(-1), minlength=NPAD).astype(np.float32)
    divisor = np.where(counts == 0.0, 1.0, counts)
    return (total / divisor[:, None])[:N].astype(np.float32)
